# revision 3
# baseline (speedup 1.0000x reference)
"""LocalContrastiveLoss Trainium2 kernel (8 cores, data-parallel over batch).

Per core (one image): load features (128,256,256) f32->bf16 into SBUF; for
each of 256 anchors, compute cosine-similarity logits against its 15x15
neighborhood via two accumulating matmuls (masked-weight trick places each
anchor's row in PSUM), then a fused softmax/positive-mean epilogue on
DVE/ACT. Host pre-computes integer label masks and does the final 16-number
reduction.
"""
import sys, os
sys.path.insert(0, "/opt/trn_rl_repo")
import numpy as np

import concourse.bass as bass
import concourse.mybir as mybir
from concourse.bass import ds
from concourse.bass_utils import run_bass_kernel_spmd

P = 128
H = W = 256
NB = 15
HALF = 7
KK = NB * NB          # 225
CTR = HALF * NB + HALF  # 112
TEMP = 0.07
NANCH = 256
NPBLK = 128           # anchors per block
NBLK = 2
NSLOT = 4


def _dyn(eng, nc, ap, lo, hi, name):
    r = eng.alloc_register(name)
    eng.reg_load(r, ap)
    return nc.s_assert_within(eng.snap(r, donate=True), lo, hi,
                              skip_runtime_assert=True)


def build(npb):
    """npb = anchors per block actually used (<=128)."""
    nc = bass.Bass()
    img_d = nc.dram_tensor("img", [P, H, W], mybir.dt.float32, kind="ExternalInput")
    offs_d = nc.dram_tensor("offs", [1, NANCH * 4], mybir.dt.int32, kind="ExternalInput")
    aux_d = nc.dram_tensor("aux", [P, 456], mybir.dt.float32, kind="ExternalInput")
    out_d = nc.dram_tensor("out", [1, 4], mybir.dt.float32, kind="ExternalOutput")

    from contextlib import ExitStack
    es = ExitStack()
    with es:
        img = es.enter_context(nc.sbuf_tensor([P, H, W], mybir.dt.bfloat16))
        offs = es.enter_context(nc.sbuf_tensor([1, NANCH * 4], mybir.dt.int32))
        aux = es.enter_context(nc.sbuf_tensor([P, 456], mybir.dt.float32))
        afm = es.enter_context(nc.sbuf_tensor([P, NSLOT * P], mybir.dt.bfloat16))
        onesm = es.enter_context(nc.sbuf_tensor([P, NSLOT * P], mybir.dt.bfloat16))
        sqs = es.enter_context(nc.sbuf_tensor([P, NSLOT * KK], mybir.dt.bfloat16))
        s_sb = es.enter_context(nc.sbuf_tensor([P, KK], mybir.dt.float32))
        n_sb = es.enter_context(nc.sbuf_tensor([P, KK], mybir.dt.float32))
        rq = es.enter_context(nc.sbuf_tensor([P, KK], mybir.dt.float32))
        pn_r = es.enter_context(nc.sbuf_tensor([P, KK], mybir.dt.float32))
        st = es.enter_context(nc.sbuf_tensor([P, KK], mybir.dt.float32))
        et = es.enter_context(nc.sbuf_tensor([P, KK], mybir.dt.float32))
        scr = es.enter_context(nc.sbuf_tensor([P, KK], mybir.dt.float32))
        sm = es.enter_context(nc.sbuf_tensor([P, 12], mybir.dt.float32))
        lcvc = es.enter_context(nc.sbuf_tensor([P, 4], mybir.dt.float32))
        ones_f = es.enter_context(nc.sbuf_tensor([P, 1], mybir.dt.float32))
        res = es.enter_context(nc.sbuf_tensor([1, 4], mybir.dt.float32))
        psum_s = es.enter_context(nc.psum_tensor([P, KK], mybir.dt.float32))
        psum_n = es.enter_context(nc.psum_tensor([P, KK], mybir.dt.float32))
        psum_f = es.enter_context(nc.psum_tensor([1, 4], mybir.dt.float32))
        dS = es.enter_context(nc.semaphore("dS"))
        sSQ = es.enter_context(nc.semaphore("sSQ"))
        sST = es.enter_context(nc.semaphore("sST"))
        sPE = es.enter_context(nc.semaphore("sPE"))
        sPC = es.enter_context(nc.semaphore("sPC"))
        sEA = es.enter_context(nc.semaphore("sEA"))
        sAE = es.enter_context(nc.semaphore("sAE"))
        sEP = es.enter_context(nc.semaphore("sEP"))
        sFR = es.enter_context(nc.semaphore("sFR"))
        sOUT = es.enter_context(nc.semaphore("sOUT"))
        block = es.enter_context(nc.Block())
        imgf = img[:].rearrange("p h w -> p (h w)")
        # column offsets in sm: 0 mneg, 1 se, 2 lnse, 3 possum, 4 pm, 5 t1, 6 li
        MNEG, SE, LNSE, PSUM_C, PM, T1 = 0, 1, 2, 3, 4, 5

        @block.gpsimd
        def _(g):
            g.dma_start(out=img[:], in_=img_d[:]).then_inc(dS, 16)
            g.dma_start(out=offs[:], in_=offs_d[:]).then_inc(dS, 16)
            g.dma_start(out=aux[:], in_=aux_d[:]).then_inc(dS, 16)
            g.wait_ge(sOUT, 1)
            g.dma_start(out=out_d[:], in_=res[:]).then_inc(dS, 16)

        @block.scalar
        def _(a):
            a.wait_ge(dS, 48)
            for b in range(NBLK):
                for i in range(npb):
                    gi = b * NPBLK + i
                    si = (b * npb + i) % NSLOT
                    if b * npb + i >= NSLOT:
                        a.wait_ge(sPE, b * npb + i - (NSLOT - 1))
                    h0 = _dyn(a, nc, offs[0:1, 4 * gi : 4 * gi + 1], 0, H - NB, f"ah{gi}")
                    w0 = _dyn(a, nc, offs[0:1, 4 * gi + 1 : 4 * gi + 2], 0, W - NB, f"aw{gi}")
                    win = img[:, ds(h0, NB), ds(w0, NB)]
                    a.square(out=sqs[:, si * KK : (si + 1) * KK], in_=win).then_inc(sSQ, 1)
                # epilogue (block b): sqrt, exp+sum, ln
                a.wait_ge(sEA, 2 * b + 1)
                a.activation(out=pn_r[:npb], in_=rq[:npb],
                             func=mybir.ActivationFunctionType.Sqrt).then_inc(sAE, 1)
                a.wait_ge(sEA, 2 * b + 2)
                a.activation(out=et[:npb], in_=st[:npb],
                             func=mybir.ActivationFunctionType.Exp,
                             bias=sm[:npb, MNEG : MNEG + 1], scale=1.0,
                             accum_out=sm[:npb, SE : SE + 1])
                a.activation(out=sm[:npb, LNSE : LNSE + 1],
                             in_=sm[:npb, SE : SE + 1],
                             func=mybir.ActivationFunctionType.Ln).then_inc(sAE, 1)

        @block.vector
        def _(v):
            v.wait_ge(dS, 48)
            v.memset(ones_f[:], 1.0)
            v.memset(lcvc[:], 0.0)
            v.tensor_copy(out=lcvc[:, 2:4], in_=aux[:, 452:454])
            for b in range(NBLK):
                for i in range(npb):
                    gi = b * NPBLK + i
                    si = (b * npb + i) % NSLOT
                    if b * npb + i >= NSLOT:
                        v.wait_ge(sPE, b * npb + i - (NSLOT - 1))
                    ctr = _dyn(v, nc, offs[0:1, 4 * gi + 2 : 4 * gi + 3], 0, H * W - 1, f"ac{gi}")
                    v.memset(afm[:, si * P : (si + 1) * P], 0.0)
                    v.tensor_copy(out=afm[:, si * P + i : si * P + i + 1],
                                  in_=imgf[:, ds(ctr, 1)])
                    v.memset(onesm[:, si * P : (si + 1) * P], 0.0)
                    v.memset(onesm[:, si * P + i : si * P + i + 1], 1.0).then_inc(sST, 1)
                # epilogue block b
                v.wait_ge(sPE, (b + 1) * npb)
                v.tensor_copy(out=s_sb[:npb], in_=psum_s[:npb])
                v.tensor_copy(out=n_sb[:npb], in_=psum_n[:npb]).then_inc(sPC, 1)
                v.reciprocal(out=rq[:npb], in_=n_sb[:npb]).then_inc(sEA, 1)
                v.wait_ge(sAE, 2 * b + 1)
                # scal = pn_r[:,CTR] * (1/TEMP)
                v.tensor_scalar_mul(out=sm[:npb, 6:7], in0=pn_r[:npb, CTR : CTR + 1],
                                    scalar1=1.0 / TEMP)
                v.tensor_tensor(out=st[:npb], in0=s_sb[:npb], in1=pn_r[:npb],
                                op=mybir.AluOpType.mult)
                v.tensor_scalar(out=st[:npb], in0=st[:npb],
                                scalar1=sm[:npb, 6:7], scalar2=None,
                                op0=mybir.AluOpType.mult)
                v.tensor_reduce(out=sm[:npb, 7:8], in_=st[:npb],
                                axis=mybir.AxisListType.X, op=mybir.AluOpType.max)
                v.tensor_scalar_mul(out=sm[:npb, MNEG : MNEG + 1],
                                    in0=sm[:npb, 7:8], scalar1=-1.0).then_inc(sEA, 1)
                # possum while ACT does exp
                v.tensor_tensor(out=scr[:npb], in0=st[:npb],
                                in1=aux[:npb, b * KK : (b + 1) * KK],
                                op=mybir.AluOpType.mult)
                v.tensor_reduce(out=sm[:npb, PSUM_C : PSUM_C + 1], in_=scr[:npb],
                                axis=mybir.AxisListType.X, op=mybir.AluOpType.add)
                v.tensor_tensor(out=sm[:npb, PM : PM + 1],
                                in0=sm[:npb, PSUM_C : PSUM_C + 1],
                                in1=aux[:npb, 450 + b : 451 + b],
                                op=mybir.AluOpType.mult)
                v.wait_ge(sAE, 2 * b + 2)
                v.tensor_tensor(out=sm[:npb, T1 : T1 + 1],
                                in0=sm[:npb, LNSE : LNSE + 1],
                                in1=sm[:npb, MNEG : MNEG + 1],
                                op=mybir.AluOpType.subtract)
                v.tensor_tensor(out=sm[:npb, T1 : T1 + 1],
                                in0=sm[:npb, T1 : T1 + 1],
                                in1=sm[:npb, PM : PM + 1],
                                op=mybir.AluOpType.subtract)
                v.tensor_tensor(out=lcvc[:npb, b : b + 1],
                                in0=sm[:npb, T1 : T1 + 1],
                                in1=aux[:npb, 452 + b : 453 + b],
                                op=mybir.AluOpType.mult).then_inc(sEP, 1)
            v.wait_ge(sFR, 1)
            v.tensor_copy(out=res[:], in_=psum_f[:]).then_inc(sOUT, 1)

        @block.tensor
        def _(t):
            t.wait_ge(dS, 48)
            for b in range(NBLK):
                if b > 0:
                    t.wait_ge(sPC, b)
                for i in range(npb):
                    gi = b * NPBLK + i
                    si = (b * npb + i) % NSLOT
                    t.wait_ge(sSQ, b * npb + i + 1)
                    t.wait_ge(sST, b * npb + i + 1)
                    h0 = _dyn(t, nc, offs[0:1, 4 * gi : 4 * gi + 1], 0, H - NB, f"th{gi}")
                    w0 = _dyn(t, nc, offs[0:1, 4 * gi + 1 : 4 * gi + 2], 0, W - NB, f"tw{gi}")
                    win = img[:, ds(h0, NB), ds(w0, NB)]
                    t.matmul(out=psum_s[:npb], lhsT=afm[:, si * P : si * P + npb],
                             rhs=win, start=(i == 0), stop=(i == npb - 1))
                    t.matmul(out=psum_n[:npb], lhsT=onesm[:, si * P : si * P + npb],
                             rhs=sqs[:, si * KK : (si + 1) * KK],
                             start=(i == 0), stop=(i == npb - 1)).then_inc(sPE, 1)
            t.wait_ge(sEP, NBLK)
            t.matmul(out=psum_f[:], lhsT=ones_f[:], rhs=lcvc[:],
                     start=True, stop=True).then_inc(sFR, 1)

    return nc


def _host_aux(targets_b, ah, aw, npb):
    """pos masks / inv counts / valid for one image; anchors in 2 blocks."""
    aux = np.zeros((P, 456), np.float32)
    for b in range(NBLK):
        for i in range(npb):
            n = b * NPBLK + i
            h, w = int(ah[n]), int(aw[n])
            L = targets_b[h - HALF : h + HALF + 1, w - HALF : w + HALF + 1]
            pos = (L == targets_b[h, w]).astype(np.float32).reshape(-1)
            pos[CTR] = 0.0
            cnt = pos.sum()
            aux[i, b * KK : (b + 1) * KK] = pos
            aux[i, 450 + b] = 1.0 / max(cnt, 1.0)
            aux[i, 452 + b] = 1.0 if cnt > 0 else 0.0
    return aux


_CACHE = {}


def kernel(features, targets, anchor_h, anchor_w):
    B = features.shape[0]
    npb = int(os.environ.get("KNPB", "128"))
    if npb not in _CACHE:
        _CACHE[npb] = build(npb)
    nc = _CACHE[npb]

    in_maps = []
    for c in range(B):
        ah, aw = anchor_h[c], anchor_w[c]
        offs = np.zeros((1, NANCH * 4), np.int32)
        for n in range(NANCH):
            offs[0, 4 * n + 0] = ah[n] - HALF
            offs[0, 4 * n + 1] = aw[n] - HALF
            offs[0, 4 * n + 2] = ah[n] * W + aw[n]
        in_maps.append(dict(
            img=np.ascontiguousarray(features[c]),
            offs=offs,
            aux=_host_aux(targets[c], ah, aw, npb),
        ))
    res = run_bass_kernel_spmd(nc, in_maps, list(range(B)))
    tot = np.float32(0.0)
    cnt = np.float32(0.0)
    for c in range(B):
        r = res.results[c]["out"].reshape(-1)
        tot += r[0] + r[1]
        cnt += r[2] + r[3]
    out = tot / max(cnt, np.float32(1.0)) if cnt > 0 else np.float32(0.0)
    return np.array(out, dtype=np.float32)
